# Initial kernel scaffold
#
"""Trainium2 Bass kernel for nn_AttentionModule_66537633349985 (segment attention pooling).

Math (per graph b): out[b] = sum_n attn_n * emb_n over nodes n with batch[n]==b,
where attn = softmax_b(w_a . tanh(W_c @ emb + b_c) + b_a). The +b_a and the
segment-max subtraction cancel in the softmax ratio, so neither is computed.

Strategy (8 NeuronCores, whole graphs per core, 128 graphs each):
  Host: pad every graph's node list to a multiple of 32 (pad nodes have
  zero embeddings, so they contribute nothing to the weighted sums),
  transpose embeddings to H-major [128, L] per core.
  Device, per 2048-node macro-tile, all in H-on-partitions layout:
    t = W_c @ embT            (fp32r matmul, W stationary)
    tT = tanh(t + b_c)        (ACT, -> bf16)
    s = w_a . tT              (matmul -> [1,512] rows at partitions 0/32/64/96)
    e = exp(s)                (ACT over the PSUM bank; junk rows unused)
    eb = ones x e_row         (PE rank-1 broadcast to all partitions)
    wE = embT * eb            (DVE)
    P = 32-block sums of wE   (DVE tensor_reduce)
    dump P [128, 64] and e [4, 512] to HBM.
  Host: per-segment sums of P blocks (reduceat), denominators from dumped
  exp(scores) of real nodes (bincount), divide, assemble [1024, 128].
"""
import numpy as np

import concourse.bass as bass
import concourse.bacc as bacc
import concourse.tile as tile
import concourse.mybir as mybir
from concourse.bass_utils import run_bass_kernel_spmd

H = 128            # hidden dim
B = 1024           # number of graphs
NCORES = 8
SPC = B // NCORES  # segments (graphs) per core
BLK = 32           # block size for on-chip partial sums; graphs padded to multiples
TM = 2048          # nodes per macro-tile
NCH = TM // 512    # 512-node chunks per macro-tile

f32 = mybir.dt.float32
f32r = mybir.dt.float32r
bf16 = mybir.dt.bfloat16

_BUILD_CACHE: dict[int, "bacc.Bacc"] = {}


def build_bass(L: int) -> "bacc.Bacc":
    """Per-core Bass program for an [H, L] H-major embedding shard."""
    if L in _BUILD_CACHE:
        return _BUILD_CACHE[L]
    assert L % TM == 0
    nmacro = L // TM

    nc = bacc.Bacc("TRN2", target_bir_lowering=False, debug=False)

    embT_d = nc.dram_tensor("embT", [H, L], f32r, kind="ExternalInput")
    W_d = nc.dram_tensor("W", [H, H], f32r, kind="ExternalInput")      # holds W_c.T
    wa_d = nc.dram_tensor("wa", [H, 1], f32, kind="ExternalInput")
    bc_d = nc.dram_tensor("bc", [H, 1], f32, kind="ExternalInput")
    P_d = nc.dram_tensor("P", [nmacro, H, TM // BLK], f32, kind="ExternalOutput")
    e_d = nc.dram_tensor("e", [nmacro, NCH, 512], f32, kind="ExternalOutput")

    Tanh = mybir.ActivationFunctionType.Tanh
    Exp = mybir.ActivationFunctionType.Exp

    with tile.TileContext(nc) as tc:
        with (
            tc.tile_pool(name="const", bufs=1) as cpool,
            tc.tile_pool(name="sbuf", bufs=2) as pool,
            tc.tile_pool(name="pt", bufs=1, space="PSUM") as pt_pool,
            tc.tile_pool(name="ps", bufs=2, space="PSUM") as ps_pool,
            tc.tile_pool(name="pe", bufs=2, space="PSUM") as pe_pool,
        ):
            W_sb = cpool.tile([H, H], f32r)
            wa_sb = cpool.tile([H, 1], f32)
            bc_sb = cpool.tile([H, 1], f32)
            wa_bf = cpool.tile([H, 1], bf16)
            ones_sb = cpool.tile([H, H], f32r)
            nc.sync.dma_start(W_sb[:], W_d[:])
            nc.sync.dma_start(wa_sb[:], wa_d[:])
            nc.sync.dma_start(bc_sb[:], bc_d[:])
            nc.vector.tensor_copy(wa_bf[:], wa_sb[:])
            nc.vector.memset(ones_sb[:].bitcast(f32), 1.0)

            for m in range(nmacro):
                emb_sb = pool.tile([H, TM], f32r, tag="emb")
                nc.sync.dma_start(emb_sb[:], embT_d[:, m * TM:(m + 1) * TM])

                # t = W_c @ embT (fp32r), 4 x N=512 into one 4-bank PSUM tile
                psum_t = pt_pool.tile([H, TM], f32, tag="pt")
                for j in range(NCH):
                    nc.tensor.matmul(
                        psum_t[:, j * 512:(j + 1) * 512],
                        W_sb[:],
                        emb_sb[:, j * 512:(j + 1) * 512],
                        start=True, stop=True,
                    )

                tT_sb = pool.tile([H, TM], bf16, tag="tT")
                nc.scalar.activation(tT_sb[:], psum_t[:], Tanh, bias=bc_sb[:])

                # s = w_a . tT -> [1, 512] rows at partitions 0/32/64/96
                psum_s = ps_pool.tile([H, 512], f32, tag="ps")
                for j in range(NCH):
                    nc.tensor.matmul(
                        psum_s[32 * j:32 * j + 1, :],
                        wa_bf[:],
                        tT_sb[:, j * 512:(j + 1) * 512],
                        start=True, stop=True,
                        tile_position=(0, 32 * j),
                    )
                e_sb = pool.tile([H, 512], f32r, tag="e")
                nc.scalar.activation(e_sb[:], psum_s[:], Exp)
                nc.sync.dma_start(e_d[m], e_sb[0:H:32, :].bitcast(f32))

                # wE = embT * exp(s) (PE rank-1 broadcast, then DVE multiply)
                wE_sb = pool.tile([H, TM], f32, tag="wE")
                for j in range(NCH):
                    psum_eb = pe_pool.tile([H, 512], f32, tag="pe")
                    nc.tensor.matmul(
                        psum_eb[:],
                        ones_sb[32 * j:32 * j + 1, :],
                        e_sb[32 * j:32 * j + 1, :],
                        start=True, stop=True,
                        tile_position=(32 * j, 0),
                    )
                    nc.vector.tensor_tensor(
                        out=wE_sb[:, j * 512:(j + 1) * 512],
                        in0=emb_sb[:, j * 512:(j + 1) * 512].bitcast(f32),
                        in1=psum_eb[:],
                        op=mybir.AluOpType.mult,
                    )

                # 32-node block partial sums
                P_sb = pool.tile([H, TM // BLK], f32, tag="P")
                nc.vector.tensor_reduce(
                    out=P_sb[:],
                    in_=wE_sb[:].rearrange("p (b k) -> p b k", k=BLK),
                    axis=mybir.AxisListType.X,
                    op=mybir.AluOpType.add,
                )
                nc.sync.dma_start(P_d[m], P_sb[:])

    nc.compile()
    _BUILD_CACHE[L] = nc
    return nc


def _prep_shards(emb: np.ndarray, batch: np.ndarray):
    """Split nodes by graph into 8 core shards; pad each graph to a multiple
    of BLK with zero-embedding nodes; return H-major shards + index metadata."""
    N = emb.shape[0]
    counts = np.bincount(batch, minlength=B).astype(np.int64)
    padded = (counts + BLK - 1) // BLK * BLK
    seg_start = np.zeros(B + 1, dtype=np.int64)
    np.cumsum(counts, out=seg_start[1:])

    core_sizes = padded.reshape(NCORES, SPC).sum(axis=1)
    L = int(np.ceil(max(core_sizes.max(), 1) / TM) * TM)

    embT = np.zeros((NCORES, H, L), dtype=np.float32)
    meta = []
    for c in range(NCORES):
        segs = np.arange(c * SPC, (c + 1) * SPC)
        cnt = counts[segs]
        pad = padded[segs]
        dst_starts = np.zeros(SPC, dtype=np.int64)
        np.cumsum(pad[:-1], out=dst_starts[1:])
        total_real = int(cnt.sum())
        # intra-segment offsets for the real nodes
        cum = np.zeros(SPC, dtype=np.int64)
        np.cumsum(cnt[:-1], out=cum[1:])
        ar = np.arange(total_real, dtype=np.int64)
        seg_ids = np.repeat(np.arange(SPC), cnt)
        intra = ar - np.repeat(cum, cnt)
        dst_pos = np.repeat(dst_starts, cnt) + intra
        src_pos = np.repeat(seg_start[segs], cnt) + intra
        embT[c][:, dst_pos] = emb[src_pos].T
        meta.append((dst_pos, seg_ids, dst_starts, pad))
    return embT, meta, L


def kernel(**inputs) -> np.ndarray:
    emb = np.ascontiguousarray(np.asarray(inputs["embeddings"], dtype=np.float32))
    batch = np.asarray(inputs["batch"]).astype(np.int64)
    W_c = np.asarray(inputs["W_c"], dtype=np.float32)
    b_c = np.asarray(inputs["b_c"], dtype=np.float32)
    w_a = np.asarray(inputs["w_a"], dtype=np.float32)
    # b_a cancels in the softmax; unused.

    embT, meta, L = _prep_shards(emb, batch)
    nc = build_bass(L)

    Wt = np.ascontiguousarray(W_c.T)
    wa_col = np.ascontiguousarray(w_a[:, None])
    bc_col = np.ascontiguousarray(b_c[:, None])
    in_maps = [
        {"embT": embT[c], "W": Wt, "wa": wa_col, "bc": bc_col}
        for c in range(NCORES)
    ]
    res = run_bass_kernel_spmd(nc, in_maps, core_ids=list(range(NCORES)))

    out = np.empty((B, H), dtype=np.float64)
    for c in range(NCORES):
        dst_pos, seg_ids, dst_starts, pad = meta[c]
        P = res.results[c]["P"]              # [nmacro, H, TM//BLK]
        e = res.results[c]["e"]              # [nmacro, NCH, 512]
        P_flat = np.moveaxis(P, 1, 0).reshape(H, -1)   # [H, L//BLK]
        e_flat = e.reshape(-1)                          # [L]
        blk_starts = (dst_starts // BLK).astype(np.int64)
        num = np.add.reduceat(P_flat.astype(np.float64), blk_starts, axis=1)  # [H, SPC]
        den = np.bincount(seg_ids, weights=e_flat[dst_pos], minlength=SPC)    # [SPC]
        out[c * SPC:(c + 1) * SPC] = (num / den[None, :]).T
    return out.astype(np.float32)


# revision 5
# speedup vs baseline: 1.4218x; 1.4218x over previous
"""Trainium2 Bass kernel for nn_AttentionModule_66537633349985 (segment attention pooling).

Math (per graph b): out[b] = sum_n attn_n * emb_n over nodes n with batch[n]==b,
where attn = softmax_b(w_a . tanh(W_c @ emb + b_c) + b_a). The +b_a and the
segment-max subtraction cancel in the softmax ratio, so neither is computed
(scores are bounded by sum|w_a| <= ~11, so exp never overflows in f32).

Sharding: nodes are split evenly across the 8 cores (125000 each, zero-padded
to 62 macro-tiles of 2048). All on-chip work is in H-on-partitions layout
[128, nodes]; the host pre-transposes embeddings once.

Device, per 2048-node macro-tile:
    t  = W_c @ embT              fp32r matmul, W stationary (4 x N=512)
    tT = tanh(t + b_c)           ACT, PSUM -> SBUF (f32r)
    s  = w_a . tT                matmul -> [1,512] rows at partitions 0/32/64/96
    e  = exp(s)                  ACT over the whole PSUM bank (junk rows unused)
    eb = ones (x) e_row          PE rank-1 broadcast to 128 partitions (f32r)
    P[:, blk] = sum(embT * eb)   fused DVE tensor_tensor_reduce per 512-chunk
    dump P [128, 4] and e [4, 512] to HBM.

Host epilogue: per-segment sums over whole 512-node blocks from P; blocks
containing a segment boundary are recomputed exactly on the host from emb and
the dumped exp(scores); denominators via bincount of dumped exp(scores);
divide and assemble the [1024, 128] output.
"""
import numpy as np

import concourse.bass as bass
import concourse.bacc as bacc
import concourse.tile as tile
import concourse.mybir as mybir
from concourse.bass_utils import run_bass_kernel_spmd

H = 128            # hidden dim
B = 1024           # number of graphs
NCORES = 8
TM = 2048          # nodes per macro-tile
NCH = TM // 512    # 512-node chunks per macro-tile
BLK = 512          # block size of the on-chip partial sums

f32 = mybir.dt.float32
f32r = mybir.dt.float32r
bf16 = mybir.dt.bfloat16

_BUILD_CACHE: dict = {}


def build_bass(L: int, repeat: int = 1) -> "bacc.Bacc":
    """Per-core Bass program for an [H, L] H-major embedding shard.

    repeat > 1 replays the whole pipeline (for marginal-time measurement);
    outputs are simply rewritten identically."""
    key = (L, repeat)
    if key in _BUILD_CACHE:
        return _BUILD_CACHE[key]
    assert L % TM == 0
    nmacro = L // TM

    nc = bacc.Bacc("TRN2", target_bir_lowering=False, debug=False)

    embT_d = nc.dram_tensor("embT", [H, L], f32r, kind="ExternalInput")
    W_d = nc.dram_tensor("W", [H, H], f32r, kind="ExternalInput")      # holds W_c.T
    wa_d = nc.dram_tensor("wa", [H, 1], f32, kind="ExternalInput")
    bc_d = nc.dram_tensor("bc", [H, 1], f32, kind="ExternalInput")
    P_d = nc.dram_tensor("P", [nmacro, H, NCH], f32, kind="ExternalOutput")
    e_d = nc.dram_tensor("e", [nmacro, NCH, 512], f32, kind="ExternalOutput")

    Tanh = mybir.ActivationFunctionType.Tanh
    Exp = mybir.ActivationFunctionType.Exp

    with tile.TileContext(nc) as tc:
        with (
            tc.tile_pool(name="const", bufs=1) as cpool,
            tc.tile_pool(name="sbuf", bufs=2) as pool,
            tc.tile_pool(name="pt", bufs=1, space="PSUM") as pt_pool,
            tc.tile_pool(name="ps", bufs=2, space="PSUM") as ps_pool,
            tc.tile_pool(name="pe", bufs=2, space="PSUM") as pe_pool,
        ):
            W_sb = cpool.tile([H, H], f32r)
            wa_sb = cpool.tile([H, 1], f32)
            wa_bf = cpool.tile([H, 1], bf16)
            bc_sb = cpool.tile([H, 1], f32)
            ones_sb = cpool.tile([H, H], f32r)
            nc.sync.dma_start(W_sb[:], W_d[:])
            nc.sync.dma_start(wa_sb[:], wa_d[:])
            nc.sync.dma_start(bc_sb[:], bc_d[:])
            nc.vector.tensor_copy(wa_bf[:], wa_sb[:])
            nc.vector.memset(ones_sb[:].bitcast(f32), 1.0)

            for m in [mm for _ in range(repeat) for mm in range(nmacro)]:
                emb_sb = pool.tile([H, TM], f32r, tag="emb")
                nc.sync.dma_start(emb_sb[:], embT_d[:, m * TM:(m + 1) * TM])

                # t = W_c @ embT (fp32r), 4 x N=512 into one 4-bank PSUM tile
                psum_t = pt_pool.tile([H, TM], f32, tag="pt")
                for j in range(NCH):
                    nc.tensor.matmul(
                        psum_t[:, j * 512:(j + 1) * 512],
                        W_sb[:],
                        emb_sb[:, j * 512:(j + 1) * 512],
                        start=True, stop=True,
                    )

                tT_sb = pool.tile([H, TM], bf16, tag="tT")
                nc.scalar.activation(tT_sb[:], psum_t[:], Tanh, bias=bc_sb[:])

                # s = w_a . tT -> [1, 512] rows at partitions 0/32/64/96
                psum_s = ps_pool.tile([H, 512], f32, tag="ps")
                for j in range(NCH):
                    nc.tensor.matmul(
                        psum_s[32 * j:32 * j + 1, :],
                        wa_bf[:],
                        tT_sb[:, j * 512:(j + 1) * 512],
                        start=True, stop=True,
                        tile_position=(0, 32 * j),
                    )
                e_sb = pool.tile([H, 512], f32r, tag="e")
                nc.scalar.activation(e_sb[:], psum_s[:], Exp)
                nc.sync.dma_start(e_d[m], e_sb[0:H:32, :].bitcast(f32))

                # wE = embT * exp(s): PE rank-1 broadcast of the e row,
                # then multiply on DVE; block sums via tensor_reduce.
                P_sb = pool.tile([H, NCH], f32, tag="P")
                wE_sb = pool.tile([H, TM], f32, tag="wE")
                for j in range(NCH):
                    psum_eb = pe_pool.tile([H, 512], f32, tag="pe")
                    nc.tensor.matmul(
                        psum_eb[:],
                        ones_sb[32 * j:32 * j + 1, :],
                        e_sb[32 * j:32 * j + 1, :],
                        start=True, stop=True,
                        tile_position=(32 * j, 0),
                    )
                    nc.vector.tensor_tensor(
                        out=wE_sb[:, j * 512:(j + 1) * 512],
                        in0=emb_sb[:, j * 512:(j + 1) * 512].bitcast(f32),
                        in1=psum_eb[:],
                        op=mybir.AluOpType.mult,
                    )
                nc.vector.tensor_reduce(
                    out=P_sb[:],
                    in_=wE_sb[:].rearrange("p (b k) -> p b k", k=BLK),
                    axis=mybir.AxisListType.X,
                    op=mybir.AluOpType.add,
                )
                nc.sync.dma_start(P_d[m], P_sb[:])

    nc.compile()
    _BUILD_CACHE[key] = nc
    return nc


def kernel(**inputs) -> np.ndarray:
    emb = np.ascontiguousarray(np.asarray(inputs["embeddings"], dtype=np.float32))
    batch = np.asarray(inputs["batch"]).astype(np.int64)
    W_c = np.asarray(inputs["W_c"], dtype=np.float32)
    b_c = np.asarray(inputs["b_c"], dtype=np.float32)
    w_a = np.asarray(inputs["w_a"], dtype=np.float32)
    # b_a cancels in the softmax; unused.

    N = emb.shape[0]
    assert N % NCORES == 0
    SH = N // NCORES                      # nodes per core
    L = ((SH + TM - 1) // TM) * TM        # zero-padded shard length

    embT = np.zeros((NCORES, H, L), dtype=np.float32)
    for c in range(NCORES):
        embT[c][:, :SH] = emb[c * SH:(c + 1) * SH].T

    nc = build_bass(L)
    Wt = np.ascontiguousarray(W_c.T)
    wa_col = np.ascontiguousarray(w_a[:, None])
    bc_col = np.ascontiguousarray(b_c[:, None])
    in_maps = [
        {"embT": embT[c], "W": Wt, "wa": wa_col, "bc": bc_col}
        for c in range(NCORES)
    ]
    res = run_bass_kernel_spmd(nc, in_maps, core_ids=list(range(NCORES)))

    num = np.zeros((B, H), dtype=np.float64)
    e_global = np.empty(N, dtype=np.float32)
    nblk_real = (SH + BLK - 1) // BLK
    for c in range(NCORES):
        P = res.results[c]["P"]                          # [nmacro, H, NCH]
        e_flat = res.results[c]["e"].reshape(-1)         # [L]
        e_global[c * SH:(c + 1) * SH] = e_flat[:SH]
        P_flat = np.moveaxis(P, 1, 0).reshape(H, -1)     # [H, L//BLK]
        for b in range(nblk_real):
            g0 = c * SH + BLK * b
            g1 = min(g0 + BLK, (c + 1) * SH)
            s0 = batch[g0]
            s1 = batch[g1 - 1]
            if s0 == s1:
                num[s0] += P_flat[:, b]
            else:
                # boundary block: recompute exactly on host per segment run
                segs = batch[g0:g1]
                eb = e_flat[BLK * b: BLK * b + (g1 - g0)].astype(np.float64)
                cuts = np.concatenate(
                    [[0], np.flatnonzero(np.diff(segs)) + 1, [g1 - g0]])
                for r in range(len(cuts) - 1):
                    r0, r1 = cuts[r], cuts[r + 1]
                    num[segs[r0]] += eb[r0:r1] @ emb[g0 + r0: g0 + r1]
    den = np.bincount(batch, weights=e_global, minlength=B)
    return (num / den[:, None]).astype(np.float32)
